# revision 29
# baseline (speedup 1.0000x reference)
"""AttentionRNN Trainium2 kernel.

Problem: B=128, T=512, H=1024, V=128
  xe = Wxh[x]                               (gather == onehot(x) @ Wxh)
  h_t = tanh(xe_t + h_{t-1} @ Whh + bh)     (512 sequential steps)
  S   = Hs @ Hs^T  (per batch);  W = softmax(S, axis=-1)
  ctx = W @ Hs;    out = [Hs, ctx] @ fc_w.T + fc_b

Sharding: data-parallel over batch, 16 batches per core, 8 cores. Params
replicated. No collectives.

Design (v2 — reworked from the 3.03ms baseline):
 - Recurrence z = h@Whh + onehot@Wxh' computed batch-major via 4 PE
   column-groups (tile_position=(0,32g)), M=16 real batches per band (no
   zero padding), full contraction accumulated in one shared PSUM tile.
 - ONE tanh ACT per step over [128, 256] (was 4 serialized ACTs at
   (256+352)/1.2 = 507ns each -> saves ~1.5us/step).
 - h transposed back hidden-major with 8 PE transposes in two
   4-concurrent waves (chunks 0,2,4,6 then 1,3,5,7 -> distinct row
   groups per wave), then 2 strided DVE copies into the ht ping-pong.
 - onehot built directly in SBUF ([128, T*16] bf16, 16KB/partition) --
   no DRAM roundtrip, no per-step DMA.
 - ~6us warm-up burst of dummy matmuls before the step loop: the
   recurrence's PE bursts (~1us) are too short to ever flip HAM to
   K=8/8, so the whole baseline recurrence ran at 1.2GHz. The burst
   warms the clock once; steady-state idle gaps (<1us) stay below the
   MID re-throttle window.
 - Attention per batch: scores via bf16 HsT, exp WITHOUT max-subtraction
   (P = exp(S) symmetric since S is), P and G in bf16.
 - ctx @ fc_wc.T = P @ (Hs @ fc_wc.T) = P @ G -- no context
   materialization. P^T blocks read from P via symmetry.
 - out[t,v] = (Hs @ fc_wh.T + 1*fc_b) + diag(1/rowsum) @ (P @ G).
"""

import os
import sys

sys.path.insert(0, "/opt/trn_rl_repo")

import numpy as np

import concourse.bass as bass
import concourse.bacc as bacc
import concourse.mybir as mybir
import concourse.tile as tile
from concourse.bass_utils import run_bass_kernel_spmd
from concourse.masks import make_identity

B, T, H, V = 128, 512, 1024, 128
NCORES = 8
BS = B // NCORES  # 16 batches per core
KCH = H // 128  # 8 hidden chunks
F32 = mybir.dt.float32
F32R = mybir.dt.float32r
BF16 = mybir.dt.bfloat16
AF = mybir.ActivationFunctionType
ALU = mybir.AluOpType

UNROLL = 16
WARMUP_MMS = 56  # ~6us of back-to-back N=128 dummy matmuls at 1.2GHz


def build_nc(t_steps=T):
    nc = bacc.Bacc(None, target_bir_lowering=False)
    n_oh_chunks = (t_steps * BS + 511) // 512  # onehot build chunks of 512 cols

    # ---- DRAM I/O ----
    whh_d = nc.dram_tensor("whh", [H, H], F32, kind="ExternalInput")
    wxhp_d = nc.dram_tensor("wxhp", [V, H], F32, kind="ExternalInput")  # Wxh + bh
    xt_d = nc.dram_tensor("xt", [n_oh_chunks, 512], F32, kind="ExternalInput")
    fcwt_d = nc.dram_tensor("fcwt", [2 * H, V], F32, kind="ExternalInput")  # fc_w.T
    fcb_d = nc.dram_tensor("fcb", [1, V], F32, kind="ExternalInput")
    out_d = nc.dram_tensor("out", [BS, t_steps, V], F32, kind="ExternalOutput")
    dbg_hst = os.environ.get("DEBUG_HST", "0") == "1"
    if dbg_hst:
        hst_d = nc.dram_tensor(
            "hstd", [128, KCH * BS * t_steps], F32, kind="ExternalOutput"
        )

    with tile.TileContext(nc) as tc:
        with tc.tile_pool(name="persist", bufs=1) as pp:
            # persistent SBUF
            hst = pp.tile([128, KCH * BS * t_steps], BF16, tag="hst")  # [p,(k b) t]
            oh_full = pp.tile([128, n_oh_chunks * 512], BF16, tag="ohfull")
            fcwt_sb = pp.tile([128, 16 * V], BF16, tag="fcwt")
            fcb_row = pp.tile([1, V], F32, tag="fcb")
            id_sb = pp.tile([128, 128], BF16, tag="ident")
            iota_f = pp.tile([128, 1], F32, tag="iotaf")
            hta = pp.tile([128, KCH * 32], BF16, tag="hta")
            htb = pp.tile([128, KCH * 32], BF16, tag="htb")

            ones_f = pp.tile([1, 128], F32, tag="onesf")
            nc.gpsimd.memset(ones_f[:], 1.0)
            id_f = pp.tile([128, 128], F32, tag="identf")
            make_identity(nc, id_f[:])
            nc.vector.tensor_copy(id_sb[:], id_f[:])
            iota_i = pp.tile([128, 1], mybir.dt.int32, tag="iotai")
            nc.gpsimd.iota(iota_i[:], pattern=[[0, 1]], base=0, channel_multiplier=1)
            nc.vector.tensor_copy(iota_f[:], iota_i[:])
            zs_f = pp.tile([128, KCH * 32], F32, tag="zsf")
            nc.gpsimd.memset(zs_f[:], 0.0)  # h0 = 0
            nc.vector.tensor_copy(hta[:], zs_f[:])
            nc.vector.tensor_copy(htb[:], zs_f[:])
            nc.gpsimd.dma_start(
                fcwt_sb.rearrange("p (c v) -> p c v", c=16)[:, :, :],
                fcwt_d.rearrange("(c p) v -> p c v", p=128)[:, :, :],
            )
            nc.gpsimd.dma_start(fcb_row[:], fcb_d[:])

            with tc.tile_pool(name="rconst", bufs=1) as rc:
                whh_sb = rc.tile([128, KCH * H], BF16, tag="whh")
                wxhp_sb = rc.tile([128, H], BF16, tag="wxhp")
                whh_raw = rc.tile([128, KCH * H], BF16, tag="whhraw")
                wxhp_raw = rc.tile([128, H], BF16, tag="wxhpraw")
                nc.gpsimd.dma_start(
                    whh_raw.rearrange("p (k h) -> p k h", k=KCH)[:, :, :],
                    whh_d.rearrange("(k p) h -> p k h", p=128)[:, :, :],
                )
                nc.gpsimd.dma_start(wxhp_raw[:], wxhp_d[:])
                nc.vector.tensor_copy(whh_sb[:], whh_raw[:])
                nc.vector.tensor_copy(wxhp_sb[:], wxhp_raw[:])

                # ---- build onehot(x) in SBUF, t-major columns (t*BS + b) ----
                with (
                    tc.tile_pool(name="ohb", bufs=3) as ohb,
                    tc.tile_pool(name="psb", bufs=2, space="PSUM") as psb,
                ):
                    for j in range(n_oh_chunks):
                        xraw = ohb.tile([1, 512], F32, tag="xraw")
                        nc.gpsimd.dma_start(xraw[:], xt_d[j : j + 1, :])
                        xrow = ohb.tile([1, 512], F32, tag="xrow")
                        nc.vector.tensor_copy(xrow[:], xraw[:])
                        psx = psb.tile([128, 512], F32, tag="psx")
                        nc.tensor.matmul(
                            psx[:], ones_f[:], xrow[:], start=True, stop=True
                        )
                        nc.vector.tensor_scalar(
                            out=oh_full[:, j * 512 : (j + 1) * 512],
                            in0=psx[:],
                            scalar1=iota_f[:],
                            scalar2=None,
                            op0=ALU.is_equal,
                        )

                # ---- recurrence ----
                with (
                    tc.tile_pool(name="ohs", bufs=2 * UNROLL) as ohs,
                    tc.tile_pool(name="hgrp", bufs=2) as hg,
                    tc.tile_pool(name="ztp", bufs=2) as ap3,
                    tc.tile_pool(name="psz", bufs=1, space="PSUM") as psz_p,
                    tc.tile_pool(name="psta", bufs=1, space="PSUM") as psta_p,
                    tc.tile_pool(name="pstb", bufs=1, space="PSUM") as pstb_p,
                    tc.tile_pool(name="pdmy", bufs=1, space="PSUM") as pdmy_p,
                ):
                    pdmy_m = pdmy_p.tile([128, 512], F32, tag="pdmym")
                    psz_ab = [
                        psz_p.tile([128, 512], F32, tag=f"psz{i}", name=f"pszs{i}")
                        for i in range(2)
                    ]

                    def pe_fence():
                        # dummy normal-mode matmul with no cross-engine deps;
                        # absorbs the PE mode-transition/structural self-wait
                        # so real matmuls keep one sync wait (S3_LW limit)
                        nc.tensor.matmul(
                            pdmy_m[0:1, 0:1], id_f[0:1, 0:1], id_f[0:1, 0:1],
                            start=True, stop=True,
                        )

                    def pe_fence_t():
                        # transpose-mode fence (before real transposes)
                        nc.tensor.transpose(
                            pdmy_m[0:1, 0:1], id_f[0:1, 0:1], id_f[0:1, 0:1]
                        )

                    # HAM warm-up: back-to-back dummy matmuls (~6us) flip the
                    # PE clock gate to K=8/8 before the step loop starts.
                    for _ in range(WARMUP_MMS):
                        nc.tensor.matmul(
                            pdmy_m[0:1, 0:128], id_f[0:1, 0:1], ones_f[0:1, :],
                            start=True, stop=True,
                        )


                    # identity slot order; transposes/copies/next-step
                    # rounds chase each band's tanh ACT (band-pair pipeline)
                    KS = [-1, 0, 1, 2, 3, 4, 5, 6, 7]
                    CPOS = {k: k for k in range(8)}

                    def step(t_expr, parity):
                        # M=32 everywhere: zero-padded lhsT cols so every psum
                        # row is matmul-written each step -- reading rows whose
                        # has_written was cleared by this group's start faults
                        # on HW (CoreSim doesn't model it)
                        oh_t = ohs.tile([128, 32], BF16, tag="oht")
                        nc.gpsimd.memset(oh_t[:, BS:32], 0.0)
                        nc.vector.tensor_copy(
                            oh_t[:, 0:BS], oh_full[:, bass.ts(t_expr, BS)]
                        )
                        ht_cur = hta if parity == 0 else htb
                        ht_new = htb if parity == 0 else hta
                        pe_fence()
                        psz = psz_ab[parity]
                        for k in KS:
                            for g in range(4):
                                if k < 0:  # vocab chunk: starts the group
                                    lhsT = oh_t[:]
                                    rhs = wxhp_sb[:, 256 * g : 256 * g + 256]
                                else:
                                    lhsT = ht_cur[
                                        :, 32 * CPOS[k] : 32 * CPOS[k] + 32
                                    ]
                                    rhs = whh_sb[
                                        :, k * H + 256 * g : k * H + 256 * g + 256
                                    ]
                                nc.tensor.matmul(
                                    psz[32 * g : 32 * g + 32, 0:256],
                                    lhsT,
                                    rhs,
                                    start=(k == -1),
                                    stop=(k == 7),
                                    tile_position=(0, 32 * g),
                                    skip_group_check=True,
                                )
                        # tanh per band (an ACT may not read psum across
                        # accumulation groups -- it faults on HW)
                        h_grp = hg.tile([128, 256], BF16, tag="zsb")
                        for g in range(4):
                            nc.scalar.activation(
                                h_grp[32 * g : 32 * g + 32, :],
                                psz[32 * g : 32 * g + 32, 0:256],
                                AF.Tanh,
                            )
                        pe_fence_t()
                        # transpose pair per band right after its tanh; pst
                        # tiles alternate psum banks so pair g's DVE copy
                        # can't collide with pair g+1's PE writes
                        ht_v = ht_new.rearrange("p (s c) -> p s c", c=32)
                        for g in range(4):
                            pool = psta_p if g % 2 == 0 else pstb_p
                            pst = pool.tile([128, 1024], BF16, tag="pst")
                            for j in range(2):
                                nc.tensor.matmul(
                                    pst[:, BS * j : BS * j + BS],
                                    h_grp[
                                        32 * g : 32 * g + BS,
                                        128 * j : 128 * j + 128,
                                    ],
                                    id_sb[
                                        32 * g : 32 * g + BS, 32 * g : 32 * g + BS
                                    ],
                                    is_transpose=True,
                                    start=True,
                                    stop=True,
                                    tile_position=(32 * g, 0),
                                )
                            nc.vector.tensor_copy(
                                ht_v[:, 2 * g : 2 * g + 2, 0:BS],
                                pst.rearrange("p (s c) -> p s c", c=BS)[
                                    :, 0:2, :
                                ],
                            )
                        # append to HsT history on gpsimd (frees DVE)
                        hst_v = hst.rearrange(
                            "p (s b t) -> p s b t", b=BS, t=t_steps
                        )
                        nc.vector.tensor_copy(
                            hst_v[:, :, :, bass.ts(t_expr, 1)],
                            ht_v[:, :, 0:BS].rearrange(
                                "p s (c one) -> p s c one", one=1
                            ),
                        )

                    if t_steps <= 32:
                        for t in range(t_steps):
                            step(t, t % 2)
                    else:
                        assert t_steps % UNROLL == 0
                        with tc.For_i(
                            0, t_steps, UNROLL, hint_engines=(mybir.EngineType.PE,)
                        ) as iv:
                            for s in range(UNROLL):
                                step(iv + s, s % 2)

            if dbg_hst:
                with tc.tile_pool(name="dbg", bufs=2) as dbgp:
                    for j in range(KCH * BS * t_steps // 512):
                        dt_ = dbgp.tile([128, 512], F32, tag="dbgt")
                        nc.vector.tensor_copy(
                            dt_[:], hst[:, j * 512 : (j + 1) * 512]
                        )
                        nc.sync.dma_start(
                            hst_d[:, j * 512 : (j + 1) * 512], dt_[:]
                        )

            # ---- attention + fc, per batch ----
            with (
                tc.tile_pool(name="attn", bufs=1) as ap_,
                tc.tile_pool(name="attn2", bufs=2) as ap2,
                tc.tile_pool(name="psS", bufs=2, space="PSUM") as psS_p,
                tc.tile_pool(name="psG", bufs=2, space="PSUM") as psG_p,
                tc.tile_pool(name="ps1", bufs=2, space="PSUM") as ps1_p,
                tc.tile_pool(name="ps2", bufs=2, space="PSUM") as ps2_p,
            ):
                hst_v = hst.rearrange("p (kb t) -> p kb t", t=t_steps)
                n_tc = t_steps // 128  # t-chunks of 128
                # hst slot s holds true hidden chunk TKS[s]; fcwt is stored in
                # true chunk order, so fc reads must map slot -> TKS[s]
                TKS = [0, 2, 4, 6, 1, 3, 5, 7]
                for b in range(BS):
                    def hs(k, sl):  # HsT tile for (k-chunk, slice of t)
                        return hst_v[:, k * BS + b, sl]

                    p_sb = ap_.tile([128, n_tc * t_steps], BF16, tag="p_sb")
                    rinv = ap_.tile([128, n_tc], F32, tag="rinv")
                    for c in range(n_tc):
                        psS = psS_p.tile([128, t_steps], F32, tag="psS")
                        for k in range(KCH):
                            nc.tensor.matmul(
                                psS[:],
                                hs(k, slice(128 * c, 128 * c + 128)),
                                hs(k, slice(0, t_steps)),
                                start=(k == 0),
                                stop=(k == KCH - 1),
                            )
                        rowsum = ap2.tile([128, 1], F32, tag="rowsum")
                        nc.scalar.activation(
                            p_sb[:, c * t_steps : (c + 1) * t_steps],
                            psS[:],
                            AF.Exp,
                            accum_out=rowsum[:],
                        )
                        nc.vector.reciprocal(rinv[:, c : c + 1], rowsum[:])
                    # G = Hs @ fc_w[:, H:].T  -> [t(=s) chunks, V]
                    g_sb = ap_.tile([128, n_tc * V], BF16, tag="g_sb")
                    for i in range(n_tc):
                        psG = psG_p.tile([128, 512], F32, tag="psG")
                        for k in range(KCH):
                            nc.tensor.matmul(
                                psG[:, 0:V],
                                hs(k, slice(128 * i, 128 * i + 128)),
                                fcwt_sb[:, (KCH + k) * V : (KCH + k + 1) * V],
                                start=(k == 0),
                                stop=(k == KCH - 1),
                            )
                        nc.vector.tensor_copy(
                            g_sb[:, i * V : (i + 1) * V], psG[:, 0:V]
                        )
                    # out[t-chunk c] = Hs@fc_wh.T + ones*fc_b + rinv*(P @ G)
                    for c in range(n_tc):
                        ps1 = ps1_p.tile([128, 512], F32, tag="ps1")
                        for k in range(KCH):
                            nc.tensor.matmul(
                                ps1[:, 0:V],
                                hs(k, slice(128 * c, 128 * c + 128)),
                                fcwt_sb[:, k * V : (k + 1) * V],
                                start=(k == 0),
                                stop=False,
                            )
                        nc.tensor.matmul(
                            ps1[:, 0:V],
                            ones_f[:],
                            fcb_row[:],
                            start=False,
                            stop=True,
                        )
                        ps2 = ps2_p.tile([128, 512], F32, tag="ps2")
                        for i in range(n_tc):
                            # lhsT = P^T block (i,c) == P block, by symmetry
                            nc.tensor.matmul(
                                ps2[:, 0:V],
                                p_sb[
                                    :,
                                    i * t_steps + 128 * c : i * t_steps + 128 * c + 128,
                                ],
                                g_sb[:, i * V : (i + 1) * V],
                                start=(i == 0),
                                stop=(i == n_tc - 1),
                            )
                        o2 = ap2.tile([128, V], F32, tag="o2")
                        nc.vector.tensor_scalar_mul(
                            o2[:], ps2[:, 0:V], rinv[:, c : c + 1]
                        )
                        oo = ap2.tile([128, V], F32, tag="oo")
                        nc.vector.tensor_add(oo[:], ps1[:, 0:V], o2[:])
                        nc.sync.dma_start(out_d[b, 128 * c : 128 * c + 128, :], oo[:])

    nc.compile()
    return nc


def _prep_core_inputs(inputs, core, t_steps=T):
    x = np.asarray(inputs["x"])[core * BS : (core + 1) * BS, :t_steps]
    wxhp = (
        np.asarray(inputs["Wxh"]).astype(np.float32)
        + np.asarray(inputs["bh"]).astype(np.float32)[None, :]
    )
    n_oh_chunks = (t_steps * BS + 511) // 512
    xt = np.zeros(n_oh_chunks * 512, dtype=np.float32)
    xt[: t_steps * BS] = x.T.reshape(-1).astype(np.float32)  # col = t*BS + b
    return {
        "whh": np.ascontiguousarray(np.asarray(inputs["Whh"]).astype(np.float32)),
        "wxhp": np.ascontiguousarray(wxhp),
        "xt": xt.reshape(n_oh_chunks, 512),
        "fcwt": np.ascontiguousarray(
            np.asarray(inputs["fc_w"]).astype(np.float32).T
        ),
        "fcb": np.asarray(inputs["fc_b"]).astype(np.float32).reshape(1, V),
    }


def kernel(x, Wxh, Whh, bh, fc_w, fc_b, t_steps=T, trace=False):
    inputs = dict(x=x, Wxh=Wxh, Whh=Whh, bh=bh, fc_w=fc_w, fc_b=fc_b)
    nc = build_nc(t_steps)
    in_maps = [_prep_core_inputs(inputs, c, t_steps) for c in range(NCORES)]
    res = run_bass_kernel_spmd(nc, in_maps, core_ids=list(range(NCORES)), trace=trace)
    out = np.concatenate([r["out"] for r in res.results], axis=0)
    if trace:
        print(f"HW exec time: {res.exec_time_ns} ns", file=sys.stderr)
    return out
